# revision 6
# baseline (speedup 1.0000x reference)
# Trainium2 Bass kernel for nn_InteractionLSTM_Cell (gnn_message_passing).
#
# Strategy: pure data-parallel over batch (B=256 -> 32 per core, 8 cores, no
# collectives).  All on-chip activations are kept "feature-major" ([feature
# partitions, row free-dim], rows = (batch, slot) pairs) so every matmul chains
# without transposes: out[f, r] = W_tile[k, f].T @ X[k, r].  All matmul
# operands are fp16 (full PE speed, ~4e-4 end-to-end rel err vs fp32), PSUM
# accumulation fp32.  The 157MB of LSTM gate weights are streamed from HBM in
# 17 double-buffered chunks overlapping the gate matmuls.  Host-side numpy
# does all layout packing (transposes / zero-padding to 128-multiples / gate
# interleave) and the inverse unpacking of outputs.
import numpy as np

# ---- problem dims (hardcoded; kernel.py must be self-contained) ----
B, T, N, D = 256, 15, 8, 64
U, HD, BUF = 2056, 512, 1680
NCORES = 8
BPC = B // NCORES            # batch per core = 32

# padded on-chip dims (multiples of 128)
ZK = 1024                    # z-input contraction rows (t*64+d, 960 -> 1024)
ZBP = 1792                   # z_b features (1680 -> 1792)
UP = 2176                    # U (2056 -> 2176)
GINP = ZBP + HD + HD + UP    # padded gate contraction = 4992
NZT = ZK // 128              # 8
NZB = ZBP // 128             # 14
NU = UP // 128               # 17
NKG = GINP // 128            # 39
NH = HD // 128               # 4

_CACHE = {}


# ---------------------------------------------------------------------------
# host-side packing helpers
# ---------------------------------------------------------------------------
def _pack_k(W, kpad, fpad=0):
    """[K, F] (np) -> [128, (kpad/128)*Fp]: k-tile t occupies cols
    [t*Fp:(t+1)*Fp], so a single contiguous DMA loads all k-tiles."""
    W = np.asarray(W)
    kp = kpad - W.shape[0]
    W = np.pad(W, ((0, kp), (0, fpad)))
    nt = kpad // 128
    return np.ascontiguousarray(
        W.reshape(nt, 128, W.shape[1]).transpose(1, 0, 2).reshape(128, -1)
    )


def _pack_bias(b, kpad):
    """[K] -> [128, kpad/128] fp32; col t = b[t*128:(t+1)*128]."""
    bp = np.pad(np.asarray(b, np.float32), (0, kpad - len(b)))
    return np.ascontiguousarray(bp.reshape(kpad // 128, 128).T)


def _prep_shared(inputs):
    f16 = np.float16
    d = {}

    # W_buf rows permuted: kernel contraction index k = t*64+d  <-  orig d*15+t
    tt, dd = np.meshgrid(np.arange(T), np.arange(D), indexing="ij")
    idx = (dd * T + tt).reshape(-1)            # idx[t*64+d] = d*15+t
    d["wbuf"] = _pack_k(np.asarray(inputs["W_buf"])[idx], ZK, fpad=ZBP - BUF).astype(f16)
    d["bbuf"] = _pack_bias(inputs["b_buf"], ZBP)

    d["wr1a"] = _pack_k(inputs["W_r1"][:U], UP).astype(f16)
    d["wr1b"] = _pack_k(inputs["W_r1"][U:], UP).astype(f16)
    d["wl1a"] = _pack_k(inputs["W_l1"][:U], UP).astype(f16)
    d["wl1b"] = _pack_k(inputs["W_l1"][U:], ZBP).astype(f16)
    d["wr2"] = _pack_k(inputs["W_r2"], HD).astype(f16)
    d["wr3"] = _pack_k(inputs["W_r3"], HD).astype(f16)
    d["wl2"] = _pack_k(inputs["W_l2"], HD).astype(f16)
    d["wl3"] = _pack_k(inputs["W_l3"], HD).astype(f16)
    d["br1"] = _pack_bias(inputs["b_r1"], HD)
    d["br2"] = _pack_bias(inputs["b_r2"], HD)
    d["br3"] = _pack_bias(inputs["b_r3"], HD)
    d["bl1"] = _pack_bias(inputs["b_l1"], HD)
    d["bl2"] = _pack_bias(inputs["b_l2"], HD)
    d["bl3"] = _pack_bias(inputs["b_l3"], HD)
    d["wd"] = _pack_k(inputs["W_d"], UP).astype(f16)
    d["bd"] = np.asarray(inputs["b_d"], np.float32).reshape(64, 1)

    # Gate weights: rows re-laid to the padded xh order
    # [zb 0:1792 | isum 1792:2304 | imax 2304:2816 | hid 2816:4992], columns
    # interleaved per 128-wide u-tile: chunk c = [I_c | F_c | O_c | C_c].
    gates = [inputs["W_I"], inputs["W_F"], inputs["W_O"], inputs["W_C"]]
    Wg = np.zeros((GINP, NU * 512), f16)
    for gi, Wn in enumerate(gates):
        Wn = np.asarray(Wn)
        Wp = np.zeros((GINP, UP), f16)
        Wp[0:1680, :U] = Wn[0:1680]
        Wp[1792:2304, :U] = Wn[1680:2192]
        Wp[2304:2816, :U] = Wn[2192:2704]
        Wp[2816:4872, :U] = Wn[2704:4760]
        for c in range(NU):
            Wg[:, c * 512 + gi * 128 : c * 512 + (gi + 1) * 128] = Wp[
                :, c * 128 : (c + 1) * 128
            ]
    d["wg"] = np.ascontiguousarray(Wg)
    bg = np.zeros((128, NU * 4), np.float32)
    for gi, bn in enumerate(
        [inputs["b_I"], inputs["b_F"], inputs["b_O"], inputs["b_C"]]
    ):
        col = _pack_bias(bn, UP)            # [128, 17]
        bg[:, gi::4] = col
    d["bg"] = bg
    return d


def _prep_core(inputs, c, bpc):
    f16 = np.float16
    r = bpc * N
    sl = slice(c * bpc, (c + 1) * bpc)
    d = {}
    z = np.asarray(inputs["z"][sl])                       # (bpc, 15, 8, 64)
    zt = z.transpose(1, 3, 0, 2).reshape(T * D, r)        # [(t,d), (b,n)]
    zt = np.pad(zt, ((0, ZK - T * D), (0, 0)))
    d["zt"] = np.ascontiguousarray(
        zt.reshape(NZT, 128, r).transpose(1, 0, 2).reshape(128, -1)
    ).astype(f16)

    def packT(M):
        Mt = np.asarray(M).reshape(bpc, N, U).transpose(2, 0, 1).reshape(U, r)
        Mt = np.pad(Mt, ((0, UP - U), (0, 0)))
        return np.ascontiguousarray(
            Mt.reshape(NU, 128, r).transpose(1, 0, 2).reshape(128, -1)
        )

    cells = packT(inputs["Cells"][sl])
    d["cells16"] = cells.astype(f16)
    d["cells32"] = cells.astype(np.float32)
    d["hid16"] = packT(inputs["hidden"][sl]).astype(f16)
    return d


# ---------------------------------------------------------------------------
# IR builder
# ---------------------------------------------------------------------------
def _build_ir(ctx, tc, aps, bpc, sim_safe=False):
    import concourse.bass as bass
    import concourse.mybir as mybir

    nc = tc.nc
    f16, f32 = mybir.dt.float16, mybir.dt.float32
    AF = mybir.ActivationFunctionType
    OP = mybir.AluOpType
    # CoreSim doesn't implement Gelu; substitute Tanh for sim-only logic tests
    GELU = AF.Tanh if sim_safe else AF.Gelu
    r = bpc * N                  # rows per core (free dim)
    rsq = bpc * N * N            # rows of the (b,i,j) tensors
    # free-dim chunking for the NxN MLP layers (PSUM bank = 512 fp32)
    CH = 512 if rsq >= 512 else rsq
    nch = (rsq + CH - 1) // CH

    pin = ctx.enter_context(tc.tile_pool(name="pin", bufs=1))
    pact = ctx.enter_context(tc.tile_pool(name="pact", bufs=1))
    tmp32 = ctx.enter_context(tc.tile_pool(name="tmp32", bufs=2))
    psA = ctx.enter_context(tc.tile_pool(name="psA", bufs=6, space="PSUM"))
    psacc = ctx.enter_context(tc.tile_pool(name="psacc", bufs=1, space="PSUM"))

    def load(name, shape, dtype):
        t = pin.tile(shape, dtype, tag=name, name=name)
        nc.sync.dma_start(t[:], aps[name][:])
        return t

    zt = load("zt", [128, NZT * r], f16)
    cells16 = load("cells16", [128, NU * r], f16)
    hid16 = load("hid16", [128, NU * r], f16)
    bbuf = load("bbuf", [128, NZB], f32)
    br1 = load("br1", [128, NH], f32)
    br2 = load("br2", [128, NH], f32)
    br3 = load("br3", [128, NH], f32)
    bl1 = load("bl1", [128, NH], f32)
    bl2 = load("bl2", [128, NH], f32)
    bl3 = load("bl3", [128, NH], f32)
    bg = load("bg", [128, NU * 4], f32)
    bd = load("bd", [64, 1], f32)
    wd = load("wd", [128, NU * 64], f16)

    zb_tiles = []
    ai_t, aj_t, al_t, bl_t = [], [], [], []

    with tc.tile_pool(name="wA", bufs=1) as wA:
        def loadw(name, shape):
            t = wA.tile(shape, f16, tag=name, name=name)
            nc.sync.dma_start(t[:], aps[name][:])
            return t

        wbuf = loadw("wbuf", [128, NZT * ZBP])
        wr1a = loadw("wr1a", [128, NU * HD])
        wr1b = loadw("wr1b", [128, NU * HD])
        wl1a = loadw("wl1a", [128, NU * HD])
        wl1b = loadw("wl1b", [128, NZB * HD])

        # ---- S1: z_b = elu(W_buf.T @ zt + b_buf), feature-major f16 tiles
        for f in range(NZB):
            ps = psA.tile([128, CH], f32, tag="ps", name=f"ps_zb{f}")
            for k in range(NZT):
                nc.tensor.matmul(
                    ps[:, :r],
                    wbuf[:, k * ZBP + f * 128 : k * ZBP + (f + 1) * 128],
                    zt[:, k * r : (k + 1) * r],
                    start=(k == 0),
                    stop=(k == NZT - 1),
                )
            # elu(x) = max(x,0) + exp(min(x,0)) - 1   (x = psum + bias)
            m = tmp32.tile([128, r], f32, tag="m", name=f"m{f}")
            nc.vector.tensor_scalar(m[:], ps[:, :r], bbuf[:, f : f + 1], 0.0,
                                    op0=OP.add, op1=OP.min)
            e = tmp32.tile([128, r], f32, tag="e", name=f"e{f}")
            nc.scalar.activation(e[:], m[:], AF.Exp)
            rl = tmp32.tile([128, r], f32, tag="rl", name=f"rl{f}")
            nc.vector.tensor_scalar(rl[:], ps[:, :r], bbuf[:, f : f + 1], 0.0,
                                    op0=OP.add, op1=OP.max)
            zbt = pact.tile([128, r], f16, tag=f"zb{f}", name=f"zb{f}")
            nc.vector.scalar_tensor_tensor(zbt[:], e[:], -1.0, rl[:],
                                           op0=OP.add, op1=OP.add)
            zb_tiles.append(zbt)

        # ---- S2: ai/aj/al (from cells), bl (from z_b) -- fp32 SBUF
        def lin4(wt, rhs_tile_fn, nk, outs, tagp, copy_eng):
            for f in range(NH):
                ps = psA.tile([128, CH], f32, tag="ps", name=f"ps_{tagp}{f}")
                for k in range(nk):
                    nc.tensor.matmul(
                        ps[:, :r],
                        wt[:, k * HD + f * 128 : k * HD + (f + 1) * 128],
                        rhs_tile_fn(k),
                        start=(k == 0),
                        stop=(k == nk - 1),
                    )
                o = pact.tile([128, r], f32, tag=f"{tagp}{f}", name=f"{tagp}{f}")
                if copy_eng == "act":
                    nc.scalar.activation(o[:], ps[:, :r], AF.Identity)
                else:
                    nc.vector.tensor_copy(o[:], ps[:, :r])
                outs.append(o)

        lin4(wr1a, lambda k: cells16[:, k * r : (k + 1) * r], NU, ai_t, "ai", "act")
        lin4(wr1b, lambda k: cells16[:, k * r : (k + 1) * r], NU, aj_t, "aj", "dve")
        lin4(wl1a, lambda k: cells16[:, k * r : (k + 1) * r], NU, al_t, "al", "act")
        lin4(wl1b, lambda k: zb_tiles[k][:], NZB, bl_t, "bl", "dve")

    # ---- S3 + S4: the two 3-layer NxN MLPs ----
    with tc.tile_pool(name="wB", bufs=1) as wB, tc.tile_pool(
        name="big", bufs=12
    ) as big:
        def loadw2(name):
            t = wB.tile([128, NH * HD], f16, tag=name, name=name)
            nc.sync.dma_start(t[:], aps[name][:])
            return t

        wr2_t, wr3_t = loadw2("wr2"), loadw2("wr3")
        wl2_t, wl3_t = loadw2("wl2"), loadw2("wl3")

        def expand_gelu(a_tiles, b_tiles, bias):
            """out[f][p, (b,i,j)] = gelu(a[p,(b,i)] + b[p,(b,j)] + bias)"""
            outs = []
            for f in range(NH):
                E = big.tile([128, rsq], f16, tag="big", name=f"E{f}")
                a_bc = a_tiles[f][:].to_broadcast((128, r, N))
                b_bc = (
                    b_tiles[f][:]
                    .rearrange("p (b j) -> p b j", j=N)
                    .unsqueeze(2)
                    .to_broadcast((128, bpc, N, N))
                )
                nc.vector.tensor_tensor(E[:], a_bc, b_bc, OP.add)
                g = big.tile([128, rsq], f16, tag="big", name=f"G{f}")
                nc.scalar.activation(g[:], E[:], GELU, bias=bias[:, f : f + 1])
                outs.append(g)
            return outs

        def mlp_layer(in_tiles, wt, bias):
            outs = []
            for f in range(NH):
                o = big.tile([128, rsq], f16, tag="big", name=f"L{f}")
                for cch in range(nch):
                    c0 = cch * CH
                    cl = min(CH, rsq - c0)
                    ps = psA.tile([128, CH], f32, tag="ps", name=f"ps_l{f}_{cch}")
                    for k in range(NH):
                        nc.tensor.matmul(
                            ps[:, :cl],
                            wt[:, k * HD + f * 128 : k * HD + (f + 1) * 128],
                            in_tiles[k][:, c0 : c0 + cl],
                            start=(k == 0),
                            stop=(k == NH - 1),
                        )
                    nc.scalar.activation(o[:, c0 : c0 + cl], ps[:, :cl], GELU,
                                         bias=bias[:, f : f + 1])
                outs.append(o)
            return outs

        r1 = expand_gelu(ai_t, aj_t, br1)
        l1 = expand_gelu(al_t, bl_t, bl1)
        r2 = mlp_layer(r1, wr2_t, br2)
        r3 = mlp_layer(r2, wr3_t, br3)
        l2 = mlp_layer(l1, wl2_t, bl2)
        l3 = mlp_layer(l2, wl3_t, bl3)

        # ---- interaction: sum_j / max_j of (r3 + l3) ----
        isum_t, imax_t = [], []
        for f in range(NH):
            it = big.tile([128, rsq], f16, tag="big", name=f"it{f}")
            nc.vector.tensor_tensor(it[:], r3[f][:], l3[f][:], OP.add)
            v = it[:].rearrange("p (c j) -> p c j", j=N)
            s32 = tmp32.tile([128, r], f32, tag="s32", name=f"s32_{f}")
            nc.vector.tensor_reduce(s32[:], v, axis=mybir.AxisListType.X, op=OP.add)
            st = pact.tile([128, r], f16, tag=f"isum{f}", name=f"isum{f}")
            nc.vector.tensor_copy(st[:], s32[:])
            isum_t.append(st)
            mt = pact.tile([128, r], f16, tag=f"imax{f}", name=f"imax{f}")
            nc.vector.tensor_reduce(mt[:], v, axis=mybir.AxisListType.X, op=OP.max)
            imax_t.append(mt)

    # ---- S5: LSTM gates, streaming W_g in NU chunks ----
    xh = (
        [t[:] for t in zb_tiles]
        + [t[:] for t in isum_t]
        + [t[:] for t in imax_t]
        + [hid16[:, k * r : (k + 1) * r] for k in range(NU)]
    )
    assert len(xh) == NKG

    wg_dram = aps["wg"]          # [GINP, NU*512]
    with tc.tile_pool(name="pwg", bufs=2) as pwg, tc.tile_pool(
        name="tmp16", bufs=3
    ) as tmp16, tc.tile_pool(name="pg1", bufs=1) as pg1:
        cells32 = pg1.tile([128, NU * r], f32, tag="cells32", name="cells32")
        nc.sync.dma_start(cells32[:], aps["cells32"][:])
        acc = psacc.tile([64, r], f32, tag="acc", name="acc")
        for c in range(NU):
            wg = pwg.tile([128, NKG * 512], f16, tag="wg", name=f"wg{c}")
            src = (
                wg_dram[:, c * 512 : (c + 1) * 512]
                .rearrange("(t p) f -> p t f", p=128)
            )
            nc.sync.dma_start(wg[:].rearrange("p (t f) -> p t f", f=512), src)
            gates = []
            for g in range(4):
                ps = psA.tile([128, CH], f32, tag="ps", name=f"ps_g{c}_{g}")
                for k in range(NKG):
                    nc.tensor.matmul(
                        ps[:, :r],
                        wg[:, k * 512 + g * 128 : k * 512 + (g + 1) * 128],
                        xh[k],
                        start=(k == 0),
                        stop=(k == NKG - 1),
                    )
                o = tmp32.tile([128, r], f32, tag=f"g{g}", name=f"g{c}_{g}")
                nc.scalar.activation(
                    o[:], ps[:, :r], AF.Tanh if g == 3 else AF.Sigmoid,
                    bias=bg[:, c * 4 + g : c * 4 + g + 1],
                )
                gates.append(o)
            I_, F_, O_, Ct_ = gates
            t1 = tmp32.tile([128, r], f32, tag="t1", name=f"t1_{c}")
            nc.vector.tensor_tensor(t1[:], F_[:], cells32[:, c * r : (c + 1) * r],
                                    OP.mult)
            t2 = tmp32.tile([128, r], f32, tag="t2", name=f"t2_{c}")
            nc.vector.tensor_tensor(t2[:], I_[:], Ct_[:], OP.mult)
            Cn = tmp32.tile([128, r], f32, tag="Cn", name=f"Cn_{c}")
            nc.vector.tensor_tensor(Cn[:], t1[:], t2[:], OP.add)
            th = tmp32.tile([128, r], f32, tag="th", name=f"th_{c}")
            nc.scalar.activation(th[:], Cn[:], AF.Tanh)
            Hn = tmp32.tile([128, r], f32, tag="Hn", name=f"Hn_{c}")
            nc.vector.tensor_tensor(Hn[:], O_[:], th[:], OP.mult)
            valid = min(128, U - c * 128)
            nc.sync.dma_start(aps["ct_out"][c * 128 : c * 128 + valid, :],
                              Cn[:valid, :])
            nc.sync.dma_start(aps["ht_out"][c * 128 : c * 128 + valid, :],
                              Hn[:valid, :])
            h16 = tmp16.tile([128, r], f16, tag="h16", name=f"h16_{c}")
            nc.vector.tensor_copy(h16[:], Hn[:])
            nc.tensor.matmul(
                acc[:], wd[:, c * 64 : (c + 1) * 64], h16[:],
                start=(c == 0), stop=(c == NU - 1),
            )
        ot = tmp32.tile([64, r], f32, tag="ot", name="ot")
        nc.scalar.activation(ot[:], acc[:], AF.Identity, bias=bd[:, 0:1])
        nc.sync.dma_start(aps["dout"][:], ot[:])


def build(bpc=BPC, num_devices=NCORES, sim_safe=False):
    """Builds and compiles the Bass module; returns nc."""
    from contextlib import ExitStack
    import concourse.mybir as mybir
    import concourse.tile as tile
    from concourse import bacc

    r = bpc * N
    nc = bacc.Bacc(
        "TRN2", target_bir_lowering=False, debug=False,
        num_devices=num_devices,
    )
    f16, f32 = mybir.dt.float16, mybir.dt.float32
    shapes = {
        "zt": ([128, NZT * r], f16),
        "cells16": ([128, NU * r], f16),
        "cells32": ([128, NU * r], f32),
        "hid16": ([128, NU * r], f16),
        "wbuf": ([128, NZT * ZBP], f16),
        "wr1a": ([128, NU * HD], f16),
        "wr1b": ([128, NU * HD], f16),
        "wl1a": ([128, NU * HD], f16),
        "wl1b": ([128, NZB * HD], f16),
        "wr2": ([128, NH * HD], f16),
        "wr3": ([128, NH * HD], f16),
        "wl2": ([128, NH * HD], f16),
        "wl3": ([128, NH * HD], f16),
        "wg": ([GINP, NU * 512], f16),
        "wd": ([128, NU * 64], f16),
        "bbuf": ([128, NZB], f32),
        "br1": ([128, NH], f32),
        "br2": ([128, NH], f32),
        "br3": ([128, NH], f32),
        "bl1": ([128, NH], f32),
        "bl2": ([128, NH], f32),
        "bl3": ([128, NH], f32),
        "bg": ([128, NU * 4], f32),
        "bd": ([64, 1], f32),
    }
    aps = {
        name: nc.dram_tensor(name, shape, dt, kind="ExternalInput").ap()
        for name, (shape, dt) in shapes.items()
    }
    aps["ct_out"] = nc.dram_tensor("ct_out", [U, r], f32, kind="ExternalOutput").ap()
    aps["ht_out"] = nc.dram_tensor("ht_out", [U, r], f32, kind="ExternalOutput").ap()
    aps["dout"] = nc.dram_tensor("dout", [64, r], f32, kind="ExternalOutput").ap()

    with tile.TileContext(nc) as tc:
        with ExitStack() as ctx:
            _build_ir(ctx, tc, aps, bpc, sim_safe=sim_safe)
    nc.compile()
    return nc


def make_in_maps(inputs, bpc=BPC, ncores=NCORES):
    shared = _prep_shared(inputs)
    in_maps = []
    for c in range(ncores):
        m = dict(shared)
        m.update(_prep_core(inputs, c, bpc))
        in_maps.append(m)
    return in_maps


def assemble(results, bpc=BPC, ncores=NCORES):
    b = bpc * ncores
    out = np.empty((b, N, D), np.float32)
    C = np.empty((b, N * U), np.float32)
    Hn = np.empty((b, N * U), np.float32)
    for c in range(ncores):
        sl = slice(c * bpc, (c + 1) * bpc)
        res = results[c]
        out[sl] = res["dout"].reshape(64, bpc, N).transpose(1, 2, 0)
        C[sl] = (
            res["ct_out"].reshape(U, bpc, N).transpose(1, 2, 0).reshape(bpc, N * U)
        )
        Hn[sl] = (
            res["ht_out"].reshape(U, bpc, N).transpose(1, 2, 0).reshape(bpc, N * U)
        )
    return out, C, Hn


def _run(inputs, trace=False, **kwargs):
    from concourse import bass_utils

    if "nc" not in _CACHE:
        _CACHE["nc"] = build()
    nc = _CACHE["nc"]
    in_maps = make_in_maps(inputs)
    res = bass_utils.run_bass_kernel_spmd(
        nc, in_maps, core_ids=list(range(NCORES)), trace=trace, **kwargs
    )
    return assemble(res.results), res


def kernel(**inputs):
    outs, _ = _run(inputs)
    return outs
